# revision 43
# baseline (speedup 1.0000x reference)
"""Trainium2 Bass kernel for nn_ComplexMixture.

Per batch element b (R = input_real[b] [S,D], I = input_imag[b] [S,D], w [S]):
    out_r = (w*R)^T R + (w*I)^T I        (symmetric)
    out_i = (w*I)^T R - (w*R)^T I        (antisymmetric)

Fold sqrt(w) into both operands (A = sqrt(w)*R, B = sqrt(w)*I) and use the
Gauss 3-multiplication complex product with E = A + B:
    M1 = A^T B,  M2 = B^T A,  M3 = E^T E
    out_r = M3 - M1 - M2
    out_i = M2 - M1
so each output block pair costs 3 PSUM-accumulated matmuls per contraction
chunk instead of 4 (25% less PE time). The combines run concurrently with
the matmul stream (only the vector engine may pair a PSUM read with a second
tensor operand; gpsimd may not touch PSUM at all):
    scalar: u = fp16(M1)                      (PSUM->SBUF copy)
    vector: t = M3 - u, out_r = t - M2, out_i = M2 - u

Sharding: data-parallel over batch, one batch element per NeuronCore (B == 8
== n_cores). Each core runs the identical program on its own slice.

Host marshalling: R/I are cast to fp16 (halves input DMA bytes) and sqrt(w)
is precomputed on host (4K scalars). Matmuls run in fp16 with fp32 PSUM
accumulation; outputs are stored as fp16 (halves store DMA bytes) and upcast
on host. Measured L2 relative error vs the fp32 reference ~5e-4.

out_r is symmetric and out_i antisymmetric, so each strictly-lower [384,384]
block is skipped on device (the pass list covers only the upper block
triangle). The host unshard mirrors them with pure transpose copies: out_r's
directly, out_i's from the device-negated oin_out.

Scheduling notes (Tile emits a static per-engine order from its own DMA
model; runtime queues are strictly in-order, and the PE HAM clock-gate
releases only after ~6us of continuous matmul activity — any gap restarts
the wait and leaves the PE at half rate):
  - inputs ride three rings in chunk order (r* on sync, i0 on scalar,
    s_t + i1-3 on gpsimd) so arrival matches consumption;
  - prep ops are spread over engines by deadline: early scales on vector,
    k2 scales on scalar, E0/E1 on gpsimd, E2/E3 back on vector;
  - the first two passes run k-major with the M3 matmuls trailing one
    chunk behind, plus dummy fillers at the chunk seams;
  - stores ride all three rings (or/sync, oi/scalar, oin/gpsimd).
"""

import sys
import types

import numpy as np

# If the environment requests tracing (BASS_TRACE=1) but the image lacks
# antenv.axon_hooks, bass_utils would crash importing it; provide a no-op
# hook registry so tracing degrades gracefully instead.
try:
    import antenv.axon_hooks  # noqa: F401
except ImportError:
    _hooks = types.ModuleType("antenv.axon_hooks")
    _hooks._hook = None
    _hooks.set_axon_ntff_profile_hook = lambda h: setattr(_hooks, "_hook", h)
    _hooks.get_axon_ntff_profile_hook = lambda: _hooks._hook
    sys.modules["antenv.axon_hooks"] = _hooks

import concourse.bacc as bacc
import concourse.bass_utils as bass_utils
import concourse.mybir as mybir
import concourse.tile as tile

B, S, D = 8, 512, 768
P = 128          # SBUF/PSUM partitions; matmul contraction tile
KC = S // P      # 4 contraction chunks per operand
MT = D // P      # 6 output row tiles
NW = 384         # matmul moving free dim (<=512 fp32 PSUM bank)
NB = D // NW     # 2 output column blocks
N_CORES = 8
N_PREWARM = 11   # dummy N=384 matmuls bridging the preamble barrier to the
                 # first real matmuls (~0.32us each at half rate)

# Upper-block-triangle passes (m row tile, col range): the strictly-lower
# 384-blocks are mirrored on host, and within each diagonal 384-block the
# lower 128-tiles are mirrors too, so those passes narrow to 256/128 cols.
PASSES = [
    (0, 0, 384),      # 0: diag1 row 0 (tiles 00,01,02)
    (0, 384, 768),    # 1: off-diag row 0
    (1, 128, 384),    # 2: diag1 row 1 (tiles 11,12)
    (1, 384, 768),    # 3: off-diag row 1
    (2, 256, 768),    # 4: diag1 row 2 + off-diag row 2, merged at N=512
    (3, 384, 768),    # 5: diag2 row 3 (tiles 33,34,35)
    (4, 512, 768),    # 6: diag2 row 4 (tiles 44,45)
    (5, 640, 768),    # 7: diag2 row 5 (tile 55)
]
PSW = 512  # PSUM tile width (one full fp32 bank; passes use [:, 0:w])
# negated out_i pieces for the host-side antisymmetric mirrors:
# pass -> (rel_lo, rel_hi, engine, target): target row of oin_out for the
# big lower-left mirror, or ('d', off) column range of the packed oind
# staging tile (stored once) for the in-diagonal-block mirrors
OIN_SPEC = {
    0: (128, 384, "g", ("d", 0)),
    1: (0, 384, "g", 0),
    2: (128, 256, "g", ("d", 256)),
    3: (0, 384, "g", 1),
    4: (128, 512, "g", 2),
    5: (128, 384, "g", ("d", 384)),
    6: (128, 256, "g", ("d", 640)),
}

_CACHE: dict = {}


def _build():
    f32, f16 = mybir.dt.float32, mybir.dt.float16
    BYP = mybir.AluOpType.bypass
    SUB = mybir.AluOpType.subtract
    nc = bacc.Bacc(
        "TRN2", target_bir_lowering=False, debug=False, num_devices=N_CORES
    )
    # Host-packed partition-major: r_in[p, k*D:(k+1)*D] = R[k*P+p, :], so a
    # whole k-chunk group is one DMA with long per-partition descriptors.
    r_d = nc.dram_tensor("r_in", [P, KC * D], f16, kind="ExternalInput").ap()
    i_d = nc.dram_tensor("i_in", [P, KC * D], f16, kind="ExternalInput").ap()
    # sqrt(w) chunks, partition-major (col k = chunk k's 128 scalars)
    s_d = nc.dram_tensor("s_in", [P, KC], f32, kind="ExternalInput").ap()
    # fused output: row r holds [out_r row | out_i row] so each pass stores
    # both halves with a single DMA trigger
    o_d = nc.dram_tensor("o_out", [D, 2, D], f16, kind="ExternalOutput").ap()
    # negated upper-right block of out_i; host transposes it into the
    # skipped lower-left block (out_i is antisymmetric)
    oin_d = nc.dram_tensor("oin_out", [D // 2, NW], f16, kind="ExternalOutput").ap()
    # negated out_i sub-tiles for the in-diagonal-block mirrors
    oind_d = nc.dram_tensor("oind_out", [P, D], f16, kind="ExternalOutput").ap()

    with tile.TileContext(nc) as tc:
        with (
            tc.tile_pool(name="const", bufs=1) as cpool,
            tc.tile_pool(name="stage", bufs=1) as spool,
            tc.tile_pool(name="abc", bufs=1) as apool,
            tc.tile_pool(name="tsb", bufs=2) as tpool,
            tc.tile_pool(name="osb", bufs=2) as opool,
            tc.tile_pool(name="ps2", bufs=2, space="PSUM") as ps2pool,
            tc.tile_pool(name="ps3", bufs=3, space="PSUM") as ps3pool,
            tc.tile_pool(name="pw", bufs=1, space="PSUM") as pwpool,
        ):
            # Scale vector first on the gpsimd ring: tiny (2KB), lands
            # before the i1-3 chunks queued behind it.
            s_t = cpool.tile([P, KC], f32, name="s_t")
            nc.gpsimd.dma_start(s_t[:], s_d)

            # packed staging for the 4 negated diag sub-tiles (stored once)
            oind_sb = cpool.tile([P, D], f16, name="oind_sb")

            # PE prewarm (see module docstring).
            zw = cpool.tile([P, 4 * P], f16, name="zw")
            nc.vector.memset(zw[:], 0.0)
            pw_ps = pwpool.tile([P, 3 * P], f32, name="pw_ps", tag="pw")
            for _ in range(N_PREWARM):
                nc.tensor.matmul(
                    pw_ps[:], zw[:, 0:P], zw[:, P : 4 * P], start=True, stop=True
                )

            def filler():
                nc.tensor.matmul(
                    pw_ps[:, 0 : 2 * P], zw[:, 0:P], zw[:, P : 3 * P],
                    start=True, stop=True,
                )

            # One k-chunk per DMA, rings loaded in consumption order.
            rt, it = [], []
            for k in range(KC):
                rt.append(spool.tile([P, D], f16, name=f"r{k}", tag=f"r{k}"))
                it.append(spool.tile([P, D], f16, name=f"i{k}", tag=f"i{k}"))

            def dsl(k):
                return slice(k * D, (k + 1) * D)

            # Inputs spread across all three rings roughly evenly, each in
            # consumption order: r* on sync, i0 on scalar, i1-3 behind s_t
            # on gpsimd. Balanced ring loads keep early arrival reliable
            # under the HBM contention of all 8 cores loading at once.
            nc.sync.dma_start(rt[0][:], r_d[:, dsl(0)])
            nc.scalar.dma_start(it[0][:], i_d[:, dsl(0)])
            nc.sync.dma_start(rt[1][:], r_d[:, dsl(1)])
            nc.gpsimd.dma_start(it[1][:], i_d[:, dsl(1)])
            nc.sync.dma_start(rt[2][:], r_d[:, dsl(2)])
            nc.gpsimd.dma_start(it[2][:], i_d[:, dsl(2)])
            nc.sync.dma_start(rt[3][:], r_d[:, dsl(3)])
            nc.gpsimd.dma_start(it[3][:], i_d[:, dsl(3)])

            # Per-row scales A/B and sums E, spread by deadline.
            At = [apool.tile([P, D], f16, name=f"A{k}", tag=f"A{k}") for k in range(KC)]
            Bt = [apool.tile([P, D], f16, name=f"B{k}", tag=f"B{k}") for k in range(KC)]
            Et = [apool.tile([P, D], f16, name=f"E{k}", tag=f"E{k}") for k in range(KC)]

            def scl(k):
                return s_t[:, k : k + 1]

            nc.vector.tensor_scalar_mul(At[0][:], rt[0][:], scl(0))
            nc.vector.tensor_scalar_mul(Bt[0][:], it[0][:], scl(0))
            nc.vector.tensor_scalar_mul(At[1][:], rt[1][:], scl(1))
            nc.vector.tensor_scalar_mul(Bt[1][:], it[1][:], scl(1))
            nc.vector.tensor_scalar_mul(At[3][:], rt[3][:], scl(3))
            nc.vector.tensor_scalar_mul(Bt[3][:], it[3][:], scl(3))
            nc.scalar.mul(At[2][:], rt[2][:], scl(2))
            nc.scalar.mul(Bt[2][:], it[2][:], scl(2))
            nc.gpsimd.tensor_add(Et[0][:], At[0][:], Bt[0][:])
            nc.gpsimd.tensor_add(Et[1][:], At[1][:], Bt[1][:])
            nc.vector.tensor_add(Et[2][:], At[2][:], Bt[2][:])
            nc.vector.tensor_add(Et[3][:], At[3][:], Bt[3][:])

            ps_of = {}

            def alloc(p):
                M1 = ps2pool.tile([P, PSW], f32, name=f"M1_{p}", tag="M1")
                M2 = ps3pool.tile([P, PSW], f32, name=f"M2_{p}", tag="M2")
                if p == len(PASSES) - 1:
                    # the prewarm bank is free by now; using it for the last
                    # (narrow, fast) pass avoids an end-of-stream bank wait
                    M3 = pwpool.tile([P, 3 * P], f32, name=f"M3_{p}", tag="pw")
                else:
                    M3 = ps2pool.tile([P, PSW], f32, name=f"M3_{p}", tag="M3")
                ps_of[p] = (M1, M2, M3)

            def mm(p, which, k, st, sp):
                m, c0, c1 = PASSES[p]
                ms = slice(m * P, (m + 1) * P)
                w = c1 - c0
                M1, M2, M3 = ps_of[p]
                dst, lt, rh = {
                    1: (M1, At[k], Bt[k]),
                    2: (M2, Bt[k], At[k]),
                    3: (M3, Et[k], Et[k]),
                }[which]
                nc.tensor.matmul(
                    dst[:, 0:w], lt[:, ms], rh[:, c0:c1], start=st, stop=sp
                )

            def evac(p):
                """Combine pass p's PSUM banks and store (fp16)."""
                m, c0, c1 = PASSES[p]
                ms = slice(m * P, (m + 1) * P)
                w = c1 - c0
                M1 = ps_of[p][0][:, 0:w]
                M2 = ps_of[p][1][:, 0:w]
                M3 = ps_of[p][2][:, 0:w]
                u = tpool.tile([P, PSW], f16, name=f"u{p}", tag="u")[:, 0:w]
                v = tpool.tile([P, PSW], f16, name=f"v{p}", tag="v")[:, 0:w]
                t = tpool.tile([P, PSW], f32, name=f"t{p}", tag="t")[:, 0:w]
                ooi = opool.tile([P, 2, PSW], f16, name=f"ooi{p}", tag="ooi")
                or_sb = ooi[:, 0, 0:w]
                oi_sb = ooi[:, 1, 0:w]
                nc.scalar.copy(u, M1)
                nc.scalar.copy(v, M2)
                nc.vector.scalar_tensor_tensor(t, M3, 0.0, u, BYP, SUB)
                nc.vector.scalar_tensor_tensor(or_sb, t, 0.0, v, BYP, SUB)
                # all-fp16 operands: DVE runs this at 2x rate
                nc.vector.tensor_sub(oi_sb, v, u)
                # one store for both output halves
                nc.sync.dma_start(o_d[ms, 0:2, c0:c1], ooi[:, 0:2, 0:w])
                # negated out_i pieces for the host-side antisymmetric
                # mirrors (exact sign flips), spread over gpsimd/vector
                if p in OIN_SPEC:
                    lo, hi, eng, tgt = OIN_SPEC[p]
                    dw = hi - lo
                    if isinstance(tgt, tuple):
                        off = tgt[1]
                        dst = oind_sb[:, off : off + dw]
                    else:
                        oin_sb = opool.tile(
                            [P, PSW], f16, name=f"oin{p}", tag="oin_sb"
                        )
                        dst = oin_sb[:, 0:dw]
                    if eng == "g":
                        nc.gpsimd.tensor_sub(dst, zw[:, 0:dw], oi_sb[:, lo:hi])
                    else:
                        nc.vector.tensor_scalar_mul(dst, oi_sb[:, lo:hi], -1.0)
                    if isinstance(tgt, tuple):
                        if p == 6:
                            # last oind contributor: one packed store
                            nc.sync.dma_start(oind_d[:, :], oind_sb[:])
                    else:
                        rr = slice(tgt * P, (tgt + 1) * P)
                        nc.sync.dma_start(oin_d[rr, :], dst)

            # Head: passes 0/1 run k-major with M3 trailing one chunk so the
            # gpsimd E sums and late chunks can't open a PE activity gap;
            # fillers pad the riskiest seams.
            alloc(0)
            alloc(1)
            for p in (0, 1):
                mm(p, 1, 0, True, False)
            for p in (0, 1):
                mm(p, 2, 0, True, False)
            filler()
            filler()
            filler()
            for p in (0, 1):
                mm(p, 1, 1, False, False)
            for p in (0, 1):
                mm(p, 2, 1, False, False)
            filler()
            for p in (0, 1):
                mm(p, 3, 0, True, False)
            for p in (0, 1):
                mm(p, 1, 2, False, False)
            for p in (0, 1):
                mm(p, 2, 2, False, False)
            filler()
            for p in (0, 1):
                mm(p, 3, 1, False, False)
            for p in (0, 1):
                mm(p, 1, 3, False, True)
            for p in (0, 1):
                mm(p, 2, 3, False, True)
            for p in (0, 1):
                mm(p, 3, 2, False, False)
            for p in (0, 1):
                mm(p, 3, 3, False, True)
            evac(0)
            evac(1)
            # Steady state: straight passes; stop group ordered M1,M3,M2 so
            # the evac chain (u needs M1, t needs M3) starts early.
            for p in range(2, len(PASSES)):
                alloc(p)
                for k in range(KC - 1):
                    for which in (1, 2, 3):
                        mm(p, which, k, k == 0, False)
                mm(p, 1, KC - 1, False, True)
                mm(p, 3, KC - 1, False, True)
                mm(p, 2, KC - 1, False, True)
                evac(p)

    nc.compile()
    return nc


def get_nc():
    if "nc" not in _CACHE:
        _CACHE["nc"] = _build()
    return _CACHE["nc"]


def make_in_maps(input_real, input_imag, weight):
    input_real = np.asarray(input_real)
    input_imag = np.asarray(input_imag)
    weight = np.asarray(weight, dtype=np.float32)
    # pack [S, D] -> [P, KC*D]: row p holds chunks k=0..KC-1 concatenated
    r16 = (
        input_real.astype(np.float16)
        .reshape(B, KC, P, D)
        .transpose(0, 2, 1, 3)
        .reshape(B, P, KC * D)
    )
    i16 = (
        input_imag.astype(np.float16)
        .reshape(B, KC, P, D)
        .transpose(0, 2, 1, 3)
        .reshape(B, P, KC * D)
    )
    # [B, P, KC]: col k = sqrt(w) for chunk k
    s_pack = np.sqrt(weight).astype(np.float32).reshape(B, KC, P).transpose(0, 2, 1)
    return [
        {
            "r_in": np.ascontiguousarray(r16[b]),
            "i_in": np.ascontiguousarray(i16[b]),
            "s_in": np.ascontiguousarray(s_pack[b]),
        }
        for b in range(B)
    ]


def unshard_single(o_np, oin_np, oind_np):
    """fp16 device outputs -> full fp32 [D,D] pair, mirroring the skipped
    strictly-lower blocks (pure transpose copies of device-computed data)."""
    o_np = np.asarray(o_np)
    out_r = o_np[:, 0, :].astype(np.float32)
    out_i = o_np[:, 1, :].astype(np.float32)
    oind = np.asarray(oind_np).astype(np.float32)
    # in-diagonal-block mirrors (out_r symmetric, out_i antisymmetric with
    # the negation already applied on device into oind)
    for b0, a_off, b_off in ((0, 0, 256), (NW, 384, 640)):
        out_r[b0 + P : b0 + NW, b0 : b0 + P] = out_r[b0 : b0 + P, b0 + P : b0 + NW].T
        out_r[b0 + 2 * P : b0 + NW, b0 + P : b0 + 2 * P] = (
            out_r[b0 + P : b0 + 2 * P, b0 + 2 * P : b0 + NW].T
        )
        out_i[b0 + P : b0 + NW, b0 : b0 + P] = oind[:, a_off : a_off + 2 * P].T
        out_i[b0 + 2 * P : b0 + NW, b0 + P : b0 + 2 * P] = (
            oind[:, b_off : b_off + P].T
        )
    # big lower-left 384-block mirrors
    out_r[NW:D, 0:NW] = out_r[0:NW, NW:D].T
    out_i[NW:D, 0:NW] = np.asarray(oin_np).astype(np.float32).T
    return out_r, out_i


def run(input_real, input_imag, weight, **spmd_kwargs):
    nc = get_nc()
    res = bass_utils.run_bass_kernel_spmd(
        nc,
        make_in_maps(input_real, input_imag, weight),
        core_ids=list(range(N_CORES)),
        **spmd_kwargs,
    )
    outs = [
        unshard_single(
            res.results[b]["o_out"], res.results[b]["oin_out"],
            res.results[b]["oind_out"],
        )
        for b in range(B)
    ]
    out_r = np.stack([o[0] for o in outs])
    out_i = np.stack([o[1] for o in outs])
    return (out_r, out_i), res


def kernel(input_real, input_imag, weight):
    (out_r, out_i), _ = run(input_real, input_imag, weight)
    return (out_r, out_i)


# revision 47
# speedup vs baseline: 1.0193x; 1.0193x over previous
"""Trainium2 Bass kernel for nn_ComplexMixture.

Per batch element b (R = input_real[b] [S,D], I = input_imag[b] [S,D], w [S]):
    out_r = (w*R)^T R + (w*I)^T I        (symmetric)
    out_i = (w*I)^T R - (w*R)^T I        (antisymmetric)

Fold sqrt(w) into both operands (A = sqrt(w)*R, B = sqrt(w)*I) and use the
Gauss 3-multiplication complex product with E = A + B:
    M1 = A^T B,  M2 = B^T A,  M3 = E^T E
    out_r = M3 - M1 - M2
    out_i = M2 - M1
so each output block pair costs 3 PSUM-accumulated matmuls per contraction
chunk instead of 4 (25% less PE time). The combines run concurrently with
the matmul stream (only the vector engine may pair a PSUM read with a second
tensor operand; gpsimd may not touch PSUM at all):
    scalar: u = fp16(M1)                      (PSUM->SBUF copy)
    vector: t = M3 - u, out_r = t - M2, out_i = M2 - u

Sharding: data-parallel over batch, one batch element per NeuronCore (B == 8
== n_cores). Each core runs the identical program on its own slice.

Host marshalling: R/I are cast to fp16 (halves input DMA bytes) and sqrt(w)
is precomputed on host (4K scalars). Matmuls run in fp16 with fp32 PSUM
accumulation; outputs are stored as fp16 (halves store DMA bytes) and upcast
on host. Measured L2 relative error vs the fp32 reference ~5e-4.

out_r is symmetric and out_i antisymmetric, so each strictly-lower [384,384]
block is skipped on device (the pass list covers only the upper block
triangle). The host unshard mirrors them with pure transpose copies: out_r's
directly, out_i's from the device-negated oin_out.

Scheduling notes (Tile emits a static per-engine order from its own DMA
model; runtime queues are strictly in-order, and the PE HAM clock-gate
releases only after ~6us of continuous matmul activity — any gap restarts
the wait and leaves the PE at half rate):
  - inputs ride three rings in chunk order (r* on sync, i0 on scalar,
    s_t + i1-3 on gpsimd) so arrival matches consumption;
  - prep ops are spread over engines by deadline: early scales on vector,
    k2 scales on scalar, E0/E1 on gpsimd, E2/E3 back on vector;
  - the first two passes run k-major with the M3 matmuls trailing one
    chunk behind, plus dummy fillers at the chunk seams;
  - stores ride all three rings (or/sync, oi/scalar, oin/gpsimd).
"""

import sys
import types

import numpy as np

# If the environment requests tracing (BASS_TRACE=1) but the image lacks
# antenv.axon_hooks, bass_utils would crash importing it; provide a no-op
# hook registry so tracing degrades gracefully instead.
try:
    import antenv.axon_hooks  # noqa: F401
except ImportError:
    _hooks = types.ModuleType("antenv.axon_hooks")
    _hooks._hook = None
    _hooks.set_axon_ntff_profile_hook = lambda h: setattr(_hooks, "_hook", h)
    _hooks.get_axon_ntff_profile_hook = lambda: _hooks._hook
    sys.modules["antenv.axon_hooks"] = _hooks

import concourse.bacc as bacc
import concourse.bass_utils as bass_utils
import concourse.mybir as mybir
import concourse.tile as tile

B, S, D = 8, 512, 768
P = 128          # SBUF/PSUM partitions; matmul contraction tile
KC = S // P      # 4 contraction chunks per operand
MT = D // P      # 6 output row tiles
NW = 384         # matmul moving free dim (<=512 fp32 PSUM bank)
NB = D // NW     # 2 output column blocks
N_CORES = 8
N_PREWARM = 9    # dummy N=384 matmuls bridging the preamble barrier to the
                 # first real matmuls (~0.32us each at half rate)

# Upper-block-triangle passes (m row tile, col range): the strictly-lower
# 384-blocks are mirrored on host, and within each diagonal 384-block the
# lower 128-tiles are mirrors too, so those passes narrow to 256/128 cols.
PASSES = [
    (0, 0, 384),      # 0: diag1 row 0 (tiles 00,01,02)
    (0, 384, 768),    # 1: off-diag row 0
    (1, 128, 384),    # 2: diag1 row 1 (tiles 11,12)
    (1, 384, 768),    # 3: off-diag row 1
    (2, 256, 768),    # 4: diag1 row 2 + off-diag row 2, merged at N=512
    (3, 384, 768),    # 5: diag2 row 3 (tiles 33,34,35)
    (4, 512, 768),    # 6: diag2 row 4 (tiles 44,45)
    (5, 640, 768),    # 7: diag2 row 5 (tile 55)
]
PSW = 512  # PSUM tile width (one full fp32 bank; passes use [:, 0:w])
# negated out_i pieces for the host-side antisymmetric mirrors:
# pass -> (rel_lo, rel_hi, engine, target): target row of oin_out for the
# big lower-left mirror, or ('d', off) column range of the packed oind
# staging tile (stored once) for the in-diagonal-block mirrors
OIN_SPEC = {
    0: (128, 384, "g", ("d", 0)),
    1: (0, 384, "g", 0),
    2: (128, 256, "g", ("d", 256)),
    3: (0, 384, "g", 1),
    4: (128, 512, "g", 2),
    5: (128, 384, "v", ("d", 384)),
    6: (128, 256, "v", ("d", 640)),
}

_CACHE: dict = {}


def _build():
    f32, f16 = mybir.dt.float32, mybir.dt.float16
    BYP = mybir.AluOpType.bypass
    SUB = mybir.AluOpType.subtract
    nc = bacc.Bacc(
        "TRN2", target_bir_lowering=False, debug=False, num_devices=N_CORES
    )
    # Host-packed partition-major: r_in[p, k*D:(k+1)*D] = R[k*P+p, :], so a
    # whole k-chunk group is one DMA with long per-partition descriptors.
    r_d = nc.dram_tensor("r_in", [P, KC * D], f16, kind="ExternalInput").ap()
    i_d = nc.dram_tensor("i_in", [P, KC * D], f16, kind="ExternalInput").ap()
    # sqrt(w) chunks, partition-major (col k = chunk k's 128 scalars)
    s_d = nc.dram_tensor("s_in", [P, KC], f32, kind="ExternalInput").ap()
    # fused output: row r holds [out_r row | out_i row] so each pass stores
    # both halves with a single DMA trigger
    o_d = nc.dram_tensor("o_out", [D, 2, D], f16, kind="ExternalOutput").ap()
    # negated upper-right block of out_i; host transposes it into the
    # skipped lower-left block (out_i is antisymmetric)
    oin_d = nc.dram_tensor("oin_out", [D // 2, NW], f16, kind="ExternalOutput").ap()
    # negated out_i sub-tiles for the in-diagonal-block mirrors
    oind_d = nc.dram_tensor("oind_out", [P, D], f16, kind="ExternalOutput").ap()

    with tile.TileContext(nc) as tc:
        with (
            tc.tile_pool(name="const", bufs=1) as cpool,
            tc.tile_pool(name="stage", bufs=1) as spool,
            tc.tile_pool(name="abc", bufs=1) as apool,
            tc.tile_pool(name="tsb", bufs=2) as tpool,
            tc.tile_pool(name="osb", bufs=2) as opool,
            tc.tile_pool(name="ps2", bufs=2, space="PSUM") as ps2pool,
            tc.tile_pool(name="ps3", bufs=3, space="PSUM") as ps3pool,
            tc.tile_pool(name="pw", bufs=1, space="PSUM") as pwpool,
        ):
            # Scale vector first on the gpsimd ring: tiny (2KB), lands
            # before the i1-3 chunks queued behind it.
            s_t = cpool.tile([P, KC], f32, name="s_t")
            nc.gpsimd.dma_start(s_t[:], s_d)

            # packed staging for the 4 negated diag sub-tiles (stored once)
            oind_sb = cpool.tile([P, D], f16, name="oind_sb")

            # PE prewarm (see module docstring).
            zw = cpool.tile([P, 4 * P], f16, name="zw")
            nc.vector.memset(zw[:], 0.0)
            pw_ps = pwpool.tile([P, 3 * P], f32, name="pw_ps", tag="pw")
            for _ in range(N_PREWARM):
                nc.tensor.matmul(
                    pw_ps[:], zw[:, 0:P], zw[:, P : 4 * P], start=True, stop=True
                )

            def filler():
                nc.tensor.matmul(
                    pw_ps[:, 0 : 2 * P], zw[:, 0:P], zw[:, P : 3 * P],
                    start=True, stop=True,
                )

            # One k-chunk per DMA, rings loaded in consumption order.
            rt, it = [], []
            for k in range(KC):
                rt.append(spool.tile([P, D], f16, name=f"r{k}", tag=f"r{k}"))
                it.append(spool.tile([P, D], f16, name=f"i{k}", tag=f"i{k}"))

            def dsl(k):
                return slice(k * D, (k + 1) * D)

            # Inputs spread across all three rings roughly evenly, each in
            # consumption order: r* on sync, i0 on scalar, i1-3 behind s_t
            # on gpsimd. Balanced ring loads keep early arrival reliable
            # under the HBM contention of all 8 cores loading at once.
            nc.sync.dma_start(rt[0][:], r_d[:, dsl(0)])
            nc.scalar.dma_start(it[0][:], i_d[:, dsl(0)])
            nc.sync.dma_start(rt[1][:], r_d[:, dsl(1)])
            nc.gpsimd.dma_start(it[1][:], i_d[:, dsl(1)])
            nc.sync.dma_start(rt[2][:], r_d[:, dsl(2)])
            nc.gpsimd.dma_start(it[2][:], i_d[:, dsl(2)])
            nc.sync.dma_start(rt[3][:], r_d[:, dsl(3)])
            nc.gpsimd.dma_start(it[3][:], i_d[:, dsl(3)])

            # Per-row scales A/B and sums E, spread by deadline.
            At = [apool.tile([P, D], f16, name=f"A{k}", tag=f"A{k}") for k in range(KC)]
            Bt = [apool.tile([P, D], f16, name=f"B{k}", tag=f"B{k}") for k in range(KC)]
            Et = [apool.tile([P, D], f16, name=f"E{k}", tag=f"E{k}") for k in range(KC)]

            def scl(k):
                return s_t[:, k : k + 1]

            nc.vector.tensor_scalar_mul(At[0][:], rt[0][:], scl(0))
            nc.vector.tensor_scalar_mul(Bt[0][:], it[0][:], scl(0))
            nc.vector.tensor_scalar_mul(At[1][:], rt[1][:], scl(1))
            nc.vector.tensor_scalar_mul(Bt[1][:], it[1][:], scl(1))
            nc.vector.tensor_scalar_mul(At[3][:], rt[3][:], scl(3))
            nc.vector.tensor_scalar_mul(Bt[3][:], it[3][:], scl(3))
            nc.scalar.mul(At[2][:], rt[2][:], scl(2))
            nc.scalar.mul(Bt[2][:], it[2][:], scl(2))
            nc.gpsimd.tensor_add(Et[0][:], At[0][:], Bt[0][:])
            nc.gpsimd.tensor_add(Et[1][:], At[1][:], Bt[1][:])
            nc.vector.tensor_add(Et[2][:], At[2][:], Bt[2][:])
            nc.vector.tensor_add(Et[3][:], At[3][:], Bt[3][:])

            ps_of = {}

            def alloc(p):
                M1 = ps2pool.tile([P, PSW], f32, name=f"M1_{p}", tag="M1")
                M2 = ps3pool.tile([P, PSW], f32, name=f"M2_{p}", tag="M2")
                if p == len(PASSES) - 1:
                    # the prewarm bank is free by now; using it for the last
                    # (narrow, fast) pass avoids an end-of-stream bank wait
                    M3 = pwpool.tile([P, 3 * P], f32, name=f"M3_{p}", tag="pw")
                else:
                    M3 = ps2pool.tile([P, PSW], f32, name=f"M3_{p}", tag="M3")
                ps_of[p] = (M1, M2, M3)

            def mm(p, which, k, st, sp):
                m, c0, c1 = PASSES[p]
                ms = slice(m * P, (m + 1) * P)
                w = c1 - c0
                M1, M2, M3 = ps_of[p]
                dst, lt, rh = {
                    1: (M1, At[k], Bt[k]),
                    2: (M2, Bt[k], At[k]),
                    3: (M3, Et[k], Et[k]),
                }[which]
                nc.tensor.matmul(
                    dst[:, 0:w], lt[:, ms], rh[:, c0:c1], start=st, stop=sp
                )

            def evac(p):
                """Combine pass p's PSUM banks and store (fp16)."""
                m, c0, c1 = PASSES[p]
                ms = slice(m * P, (m + 1) * P)
                w = c1 - c0
                M1 = ps_of[p][0][:, 0:w]
                M2 = ps_of[p][1][:, 0:w]
                M3 = ps_of[p][2][:, 0:w]
                u = tpool.tile([P, PSW], f16, name=f"u{p}", tag="u")[:, 0:w]
                t = tpool.tile([P, PSW], f32, name=f"t{p}", tag="t")[:, 0:w]
                ooi = opool.tile([P, 2, PSW], f16, name=f"ooi{p}", tag="ooi")
                or_sb = ooi[:, 0, 0:w]
                oi_sb = ooi[:, 1, 0:w]
                nc.scalar.copy(u, M1)
                nc.vector.scalar_tensor_tensor(t, M3, 0.0, u, BYP, SUB)
                nc.vector.scalar_tensor_tensor(or_sb, t, 0.0, M2, BYP, SUB)
                nc.vector.scalar_tensor_tensor(oi_sb, M2, 0.0, u, BYP, SUB)
                # one store for both output halves
                nc.sync.dma_start(o_d[ms, 0:2, c0:c1], ooi[:, 0:2, 0:w])
                # negated out_i pieces for the host-side antisymmetric
                # mirrors (exact sign flips), spread over gpsimd/vector
                if p in OIN_SPEC:
                    lo, hi, eng, tgt = OIN_SPEC[p]
                    dw = hi - lo
                    if isinstance(tgt, tuple):
                        off = tgt[1]
                        dst = oind_sb[:, off : off + dw]
                    else:
                        oin_sb = opool.tile(
                            [P, PSW], f16, name=f"oin{p}", tag="oin_sb"
                        )
                        dst = oin_sb[:, 0:dw]
                    if eng == "g":
                        nc.gpsimd.tensor_sub(dst, zw[:, 0:dw], oi_sb[:, lo:hi])
                    else:
                        nc.vector.tensor_scalar_mul(dst, oi_sb[:, lo:hi], -1.0)
                    if isinstance(tgt, tuple):
                        if p == 6:
                            # last oind contributor: one packed store
                            nc.sync.dma_start(oind_d[:, :], oind_sb[:])
                    else:
                        rr = slice(tgt * P, (tgt + 1) * P)
                        nc.sync.dma_start(oin_d[rr, :], dst)

            # Head: passes 0/1 run k-major with M3 trailing one chunk so the
            # gpsimd E sums and late chunks can't open a PE activity gap;
            # fillers pad the riskiest seams.
            alloc(0)
            alloc(1)
            for p in (0, 1):
                mm(p, 1, 0, True, False)
            for p in (0, 1):
                mm(p, 2, 0, True, False)
            filler()
            for p in (0, 1):
                mm(p, 1, 1, False, False)
            for p in (0, 1):
                mm(p, 2, 1, False, False)
            filler()
            for p in (0, 1):
                mm(p, 3, 0, True, False)
            for p in (0, 1):
                mm(p, 1, 2, False, False)
            for p in (0, 1):
                mm(p, 2, 2, False, False)
            filler()
            for p in (0, 1):
                mm(p, 3, 1, False, False)
            for p in (0, 1):
                mm(p, 1, 3, False, True)
            for p in (0, 1):
                mm(p, 2, 3, False, True)
            for p in (0, 1):
                mm(p, 3, 2, False, False)
            for p in (0, 1):
                mm(p, 3, 3, False, True)
            evac(0)
            evac(1)
            # Steady state: straight passes; stop group ordered M1,M3,M2 so
            # the evac chain (u needs M1, t needs M3) starts early.
            for p in range(2, len(PASSES)):
                alloc(p)
                for k in range(KC - 1):
                    for which in (1, 2, 3):
                        mm(p, which, k, k == 0, False)
                mm(p, 1, KC - 1, False, True)
                mm(p, 3, KC - 1, False, True)
                mm(p, 2, KC - 1, False, True)
                evac(p)

    nc.compile()
    return nc


def get_nc():
    if "nc" not in _CACHE:
        _CACHE["nc"] = _build()
    return _CACHE["nc"]


def make_in_maps(input_real, input_imag, weight):
    input_real = np.asarray(input_real)
    input_imag = np.asarray(input_imag)
    weight = np.asarray(weight, dtype=np.float32)
    # pack [S, D] -> [P, KC*D]: row p holds chunks k=0..KC-1 concatenated
    r16 = (
        input_real.astype(np.float16)
        .reshape(B, KC, P, D)
        .transpose(0, 2, 1, 3)
        .reshape(B, P, KC * D)
    )
    i16 = (
        input_imag.astype(np.float16)
        .reshape(B, KC, P, D)
        .transpose(0, 2, 1, 3)
        .reshape(B, P, KC * D)
    )
    # [B, P, KC]: col k = sqrt(w) for chunk k
    s_pack = np.sqrt(weight).astype(np.float32).reshape(B, KC, P).transpose(0, 2, 1)
    return [
        {
            "r_in": np.ascontiguousarray(r16[b]),
            "i_in": np.ascontiguousarray(i16[b]),
            "s_in": np.ascontiguousarray(s_pack[b]),
        }
        for b in range(B)
    ]


def unshard_single(o_np, oin_np, oind_np):
    """fp16 device outputs -> full fp32 [D,D] pair, mirroring the skipped
    strictly-lower blocks (pure transpose copies of device-computed data)."""
    o_np = np.asarray(o_np)
    out_r = o_np[:, 0, :].astype(np.float32)
    out_i = o_np[:, 1, :].astype(np.float32)
    oind = np.asarray(oind_np).astype(np.float32)
    # in-diagonal-block mirrors (out_r symmetric, out_i antisymmetric with
    # the negation already applied on device into oind)
    for b0, a_off, b_off in ((0, 0, 256), (NW, 384, 640)):
        out_r[b0 + P : b0 + NW, b0 : b0 + P] = out_r[b0 : b0 + P, b0 + P : b0 + NW].T
        out_r[b0 + 2 * P : b0 + NW, b0 + P : b0 + 2 * P] = (
            out_r[b0 + P : b0 + 2 * P, b0 + 2 * P : b0 + NW].T
        )
        out_i[b0 + P : b0 + NW, b0 : b0 + P] = oind[:, a_off : a_off + 2 * P].T
        out_i[b0 + 2 * P : b0 + NW, b0 + P : b0 + 2 * P] = (
            oind[:, b_off : b_off + P].T
        )
    # big lower-left 384-block mirrors
    out_r[NW:D, 0:NW] = out_r[0:NW, NW:D].T
    out_i[NW:D, 0:NW] = np.asarray(oin_np).astype(np.float32).T
    return out_r, out_i


def run(input_real, input_imag, weight, **spmd_kwargs):
    nc = get_nc()
    res = bass_utils.run_bass_kernel_spmd(
        nc,
        make_in_maps(input_real, input_imag, weight),
        core_ids=list(range(N_CORES)),
        **spmd_kwargs,
    )
    outs = [
        unshard_single(
            res.results[b]["o_out"], res.results[b]["oin_out"],
            res.results[b]["oind_out"],
        )
        for b in range(B)
    ]
    out_r = np.stack([o[0] for o in outs])
    out_i = np.stack([o[1] for o in outs])
    return (out_r, out_i), res


def kernel(input_real, input_imag, weight):
    (out_r, out_i), _ = run(input_real, input_imag, weight)
    return (out_r, out_i)


# revision 48
# speedup vs baseline: 1.0536x; 1.0337x over previous
"""Trainium2 Bass kernel for nn_ComplexMixture.

Per batch element b (R = input_real[b] [S,D], I = input_imag[b] [S,D], w [S]):
    out_r = (w*R)^T R + (w*I)^T I        (symmetric)
    out_i = (w*I)^T R - (w*R)^T I        (antisymmetric)

Fold sqrt(w) into both operands (A = sqrt(w)*R, B = sqrt(w)*I) and use the
Gauss 3-multiplication complex product with E = A + B:
    M1 = A^T B,  M2 = B^T A,  M3 = E^T E
    out_r = M3 - M1 - M2
    out_i = M2 - M1
so each output block pair costs 3 PSUM-accumulated matmuls per contraction
chunk instead of 4 (25% less PE time). The combines run concurrently with
the matmul stream (only the vector engine may pair a PSUM read with a second
tensor operand; gpsimd may not touch PSUM at all):
    scalar: u = fp16(M1)                      (PSUM->SBUF copy)
    vector: t = M3 - u, out_r = t - M2, out_i = M2 - u

Sharding: data-parallel over batch, one batch element per NeuronCore (B == 8
== n_cores). Each core runs the identical program on its own slice.

Host marshalling: R/I are cast to fp16 (halves input DMA bytes) and sqrt(w)
is precomputed on host (4K scalars). Matmuls run in fp16 with fp32 PSUM
accumulation; outputs are stored as fp16 (halves store DMA bytes) and upcast
on host. Measured L2 relative error vs the fp32 reference ~5e-4.

out_r is symmetric and out_i antisymmetric, so each strictly-lower [384,384]
block is skipped on device (the pass list covers only the upper block
triangle). The host unshard mirrors them with pure transpose copies: out_r's
directly, out_i's from the device-negated oin_out.

Scheduling notes (Tile emits a static per-engine order from its own DMA
model; runtime queues are strictly in-order, and the PE HAM clock-gate
releases only after ~6us of continuous matmul activity — any gap restarts
the wait and leaves the PE at half rate):
  - inputs ride three rings in chunk order (r* on sync, i0 on scalar,
    s_t + i1-3 on gpsimd) so arrival matches consumption;
  - prep ops are spread over engines by deadline: early scales on vector,
    k2 scales on scalar, E0/E1 on gpsimd, E2/E3 back on vector;
  - the first two passes run k-major with the M3 matmuls trailing one
    chunk behind, plus dummy fillers at the chunk seams;
  - stores ride all three rings (or/sync, oi/scalar, oin/gpsimd).
"""

import sys
import types

import numpy as np

# If the environment requests tracing (BASS_TRACE=1) but the image lacks
# antenv.axon_hooks, bass_utils would crash importing it; provide a no-op
# hook registry so tracing degrades gracefully instead.
try:
    import antenv.axon_hooks  # noqa: F401
except ImportError:
    _hooks = types.ModuleType("antenv.axon_hooks")
    _hooks._hook = None
    _hooks.set_axon_ntff_profile_hook = lambda h: setattr(_hooks, "_hook", h)
    _hooks.get_axon_ntff_profile_hook = lambda: _hooks._hook
    sys.modules["antenv.axon_hooks"] = _hooks

import concourse.bacc as bacc
import concourse.bass_utils as bass_utils
import concourse.mybir as mybir
import concourse.tile as tile

B, S, D = 8, 512, 768
P = 128          # SBUF/PSUM partitions; matmul contraction tile
KC = S // P      # 4 contraction chunks per operand
MT = D // P      # 6 output row tiles
NW = 384         # matmul moving free dim (<=512 fp32 PSUM bank)
NB = D // NW     # 2 output column blocks
N_CORES = 8
N_PREWARM = 9    # dummy N=384 matmuls bridging the preamble barrier to the
                 # first real matmuls (~0.32us each at half rate)

# Upper-block-triangle passes (m row tile, col range): the strictly-lower
# 384-blocks are mirrored on host, and within each diagonal 384-block the
# lower 128-tiles are mirrors too, so those passes narrow to 256/128 cols.
PASSES = [
    (0, 0, 384),      # 0: diag1 row 0 (tiles 00,01,02)
    (0, 384, 768),    # 1: off-diag row 0
    (1, 128, 384),    # 2: diag1 row 1 (tiles 11,12)
    (1, 384, 768),    # 3: off-diag row 1
    (2, 256, 768),    # 4: diag1 row 2 + off-diag row 2, merged at N=512
    (3, 384, 768),    # 5: diag2 row 3 (tiles 33,34,35)
    (4, 512, 768),    # 6: diag2 row 4 (tiles 44,45)
    (5, 640, 768),    # 7: diag2 row 5 (tile 55)
]
PSW = 512  # PSUM tile width (one full fp32 bank; passes use [:, 0:w])
# negated out_i pieces for the host-side antisymmetric mirrors:
# pass -> (rel_lo, rel_hi, engine, target): target row of oin_out for the
# big lower-left mirror, or ('d', off) column range of the packed oind
# staging tile (stored once) for the in-diagonal-block mirrors
OIN_SPEC = {
    0: (128, 384, "g", ("d", 0)),
    1: (0, 384, "g", 0),
    2: (128, 256, "g", ("d", 256)),
    3: (0, 384, "g", 1),
    4: (128, 512, "g", 2),
    5: (128, 384, "v", ("d", 384)),
    6: (128, 256, "v", ("d", 640)),
}

_CACHE: dict = {}


def _build():
    f32, f16 = mybir.dt.float32, mybir.dt.float16
    BYP = mybir.AluOpType.bypass
    SUB = mybir.AluOpType.subtract
    nc = bacc.Bacc(
        "TRN2", target_bir_lowering=False, debug=False, num_devices=N_CORES
    )
    # Host-packed partition-major: r_in[p, k*D:(k+1)*D] = R[k*P+p, :], so a
    # whole k-chunk group is one DMA with long per-partition descriptors.
    r_d = nc.dram_tensor("r_in", [P, KC * D], f16, kind="ExternalInput").ap()
    i_d = nc.dram_tensor("i_in", [P, KC * D], f16, kind="ExternalInput").ap()
    # sqrt(w) chunks, partition-major (col k = chunk k's 128 scalars)
    s_d = nc.dram_tensor("s_in", [P, KC], f32, kind="ExternalInput").ap()
    # fused output: row r holds [out_r row | out_i row] so each pass stores
    # both halves with a single DMA trigger
    o_d = nc.dram_tensor("o_out", [D, 2, D], f16, kind="ExternalOutput").ap()
    # negated upper-right block of out_i; host transposes it into the
    # skipped lower-left block (out_i is antisymmetric)
    oin_d = nc.dram_tensor("oin_out", [D // 2, NW], f16, kind="ExternalOutput").ap()
    # negated out_i sub-tiles for the in-diagonal-block mirrors
    oind_d = nc.dram_tensor("oind_out", [P, D], f16, kind="ExternalOutput").ap()

    with tile.TileContext(nc) as tc:
        with (
            tc.tile_pool(name="const", bufs=1) as cpool,
            tc.tile_pool(name="stage", bufs=1) as spool,
            tc.tile_pool(name="abc", bufs=1) as apool,
            tc.tile_pool(name="tsb", bufs=2) as tpool,
            tc.tile_pool(name="osb", bufs=2) as opool,
            tc.tile_pool(name="ps2", bufs=2, space="PSUM") as ps2pool,
            tc.tile_pool(name="ps3", bufs=3, space="PSUM") as ps3pool,
            tc.tile_pool(name="pw", bufs=1, space="PSUM") as pwpool,
        ):
            # Scale vector first on the gpsimd ring: tiny (2KB), lands
            # before the i1-3 chunks queued behind it.
            s_t = cpool.tile([P, KC], f32, name="s_t")
            nc.gpsimd.dma_start(s_t[:], s_d)

            # packed staging for the 4 negated diag sub-tiles (stored once)
            oind_sb = cpool.tile([P, D], f16, name="oind_sb")

            # PE prewarm (see module docstring).
            zw = cpool.tile([P, 4 * P], f16, name="zw")
            nc.vector.memset(zw[:], 0.0)
            pw_ps = pwpool.tile([P, 3 * P], f32, name="pw_ps", tag="pw")
            for _ in range(N_PREWARM):
                nc.tensor.matmul(
                    pw_ps[:], zw[:, 0:P], zw[:, P : 4 * P], start=True, stop=True
                )

            def filler():
                nc.tensor.matmul(
                    pw_ps[:, 0 : 2 * P], zw[:, 0:P], zw[:, P : 3 * P],
                    start=True, stop=True,
                )

            # One k-chunk per DMA, rings loaded in consumption order.
            rt, it = [], []
            for k in range(KC):
                rt.append(spool.tile([P, D], f16, name=f"r{k}", tag=f"r{k}"))
                it.append(spool.tile([P, D], f16, name=f"i{k}", tag=f"i{k}"))

            def dsl(k):
                return slice(k * D, (k + 1) * D)

            # Inputs spread across all three rings roughly evenly, each in
            # consumption order: r* on sync, i0 on scalar, i1-3 behind s_t
            # on gpsimd. Balanced ring loads keep early arrival reliable
            # under the HBM contention of all 8 cores loading at once.
            nc.sync.dma_start(rt[0][:], r_d[:, dsl(0)])
            nc.scalar.dma_start(it[0][:], i_d[:, dsl(0)])
            nc.sync.dma_start(rt[1][:], r_d[:, dsl(1)])
            nc.gpsimd.dma_start(it[1][:], i_d[:, dsl(1)])
            nc.sync.dma_start(rt[2][:], r_d[:, dsl(2)])
            nc.gpsimd.dma_start(it[2][:], i_d[:, dsl(2)])
            nc.sync.dma_start(rt[3][:], r_d[:, dsl(3)])
            nc.gpsimd.dma_start(it[3][:], i_d[:, dsl(3)])

            # Per-row scales A/B and sums E, spread by deadline.
            At = [apool.tile([P, D], f16, name=f"A{k}", tag=f"A{k}") for k in range(KC)]
            Bt = [apool.tile([P, D], f16, name=f"B{k}", tag=f"B{k}") for k in range(KC)]
            Et = [apool.tile([P, D], f16, name=f"E{k}", tag=f"E{k}") for k in range(KC)]

            def scl(k):
                return s_t[:, k : k + 1]

            nc.vector.tensor_scalar_mul(At[0][:], rt[0][:], scl(0))
            nc.vector.tensor_scalar_mul(Bt[0][:], it[0][:], scl(0))
            nc.vector.tensor_scalar_mul(At[1][:], rt[1][:], scl(1))
            nc.vector.tensor_scalar_mul(Bt[1][:], it[1][:], scl(1))
            nc.vector.tensor_scalar_mul(At[3][:], rt[3][:], scl(3))
            nc.vector.tensor_scalar_mul(Bt[3][:], it[3][:], scl(3))
            nc.scalar.mul(At[2][:], rt[2][:], scl(2))
            nc.scalar.mul(Bt[2][:], it[2][:], scl(2))
            nc.gpsimd.tensor_add(Et[0][:], At[0][:], Bt[0][:])
            nc.gpsimd.tensor_add(Et[1][:], At[1][:], Bt[1][:])
            nc.vector.tensor_add(Et[2][:], At[2][:], Bt[2][:])
            nc.vector.tensor_add(Et[3][:], At[3][:], Bt[3][:])

            ps_of = {}

            def alloc(p):
                M1 = ps2pool.tile([P, PSW], f32, name=f"M1_{p}", tag="M1")
                M2 = ps3pool.tile([P, PSW], f32, name=f"M2_{p}", tag="M2")
                if p == len(PASSES) - 1:
                    # the prewarm bank is free by now; using it for the last
                    # (narrow, fast) pass avoids an end-of-stream bank wait
                    M3 = pwpool.tile([P, 3 * P], f32, name=f"M3_{p}", tag="pw")
                else:
                    M3 = ps2pool.tile([P, PSW], f32, name=f"M3_{p}", tag="M3")
                ps_of[p] = (M1, M2, M3)

            def mm(p, which, k, st, sp):
                m, c0, c1 = PASSES[p]
                ms = slice(m * P, (m + 1) * P)
                w = c1 - c0
                M1, M2, M3 = ps_of[p]
                dst, lt, rh = {
                    1: (M1, At[k], Bt[k]),
                    2: (M2, Bt[k], At[k]),
                    3: (M3, Et[k], Et[k]),
                }[which]
                nc.tensor.matmul(
                    dst[:, 0:w], lt[:, ms], rh[:, c0:c1], start=st, stop=sp
                )

            def evac(p):
                """Combine pass p's PSUM banks and store (fp16)."""
                m, c0, c1 = PASSES[p]
                ms = slice(m * P, (m + 1) * P)
                w = c1 - c0
                M1 = ps_of[p][0][:, 0:w]
                M2 = ps_of[p][1][:, 0:w]
                M3 = ps_of[p][2][:, 0:w]
                u = tpool.tile([P, PSW], f16, name=f"u{p}", tag="u")[:, 0:w]
                v = tpool.tile([P, PSW], f16, name=f"v{p}", tag="v")[:, 0:w]
                t = tpool.tile([P, PSW], f16, name=f"t{p}", tag="t")[:, 0:w]
                ooi = opool.tile([P, 2, PSW], f16, name=f"ooi{p}", tag="ooi")
                or_sb = ooi[:, 0, 0:w]
                oi_sb = ooi[:, 1, 0:w]
                # scalar (the only engine with slack) drains all three PSUM
                # banks to fp16; every combine below is then a cheap 2x-rate
                # all-fp16 vector op
                nc.scalar.copy(u, M1)
                nc.scalar.copy(v, M2)
                nc.scalar.copy(t, M3)
                nc.vector.tensor_sub(or_sb, t, u)
                nc.vector.tensor_sub(or_sb, or_sb, v)
                nc.vector.tensor_sub(oi_sb, v, u)
                # one store for both output halves
                nc.sync.dma_start(o_d[ms, 0:2, c0:c1], ooi[:, 0:2, 0:w])
                # negated out_i pieces for the host-side antisymmetric
                # mirrors (exact sign flips), spread over gpsimd/vector
                if p in OIN_SPEC:
                    lo, hi, eng, tgt = OIN_SPEC[p]
                    dw = hi - lo
                    if isinstance(tgt, tuple):
                        off = tgt[1]
                        dst = oind_sb[:, off : off + dw]
                    else:
                        oin_sb = opool.tile(
                            [P, PSW], f16, name=f"oin{p}", tag="oin_sb"
                        )
                        dst = oin_sb[:, 0:dw]
                    if eng == "g":
                        nc.gpsimd.tensor_sub(dst, zw[:, 0:dw], oi_sb[:, lo:hi])
                    else:
                        nc.vector.tensor_scalar_mul(dst, oi_sb[:, lo:hi], -1.0)
                    if isinstance(tgt, tuple):
                        if p == 6:
                            # last oind contributor: one packed store
                            nc.sync.dma_start(oind_d[:, :], oind_sb[:])
                    else:
                        rr = slice(tgt * P, (tgt + 1) * P)
                        nc.sync.dma_start(oin_d[rr, :], dst)

            # Head: passes 0/1 run k-major with M3 trailing one chunk so the
            # gpsimd E sums and late chunks can't open a PE activity gap;
            # fillers pad the riskiest seams.
            alloc(0)
            alloc(1)
            for p in (0, 1):
                mm(p, 1, 0, True, False)
            for p in (0, 1):
                mm(p, 2, 0, True, False)
            filler()
            for p in (0, 1):
                mm(p, 1, 1, False, False)
            for p in (0, 1):
                mm(p, 2, 1, False, False)
            filler()
            for p in (0, 1):
                mm(p, 3, 0, True, False)
            for p in (0, 1):
                mm(p, 1, 2, False, False)
            for p in (0, 1):
                mm(p, 2, 2, False, False)
            filler()
            for p in (0, 1):
                mm(p, 3, 1, False, False)
            for p in (0, 1):
                mm(p, 1, 3, False, True)
            for p in (0, 1):
                mm(p, 2, 3, False, True)
            for p in (0, 1):
                mm(p, 3, 2, False, False)
            for p in (0, 1):
                mm(p, 3, 3, False, True)
            evac(0)
            evac(1)
            # Steady state: straight passes; stop group ordered M1,M3,M2 so
            # the evac chain (u needs M1, t needs M3) starts early.
            for p in range(2, len(PASSES)):
                alloc(p)
                for k in range(KC - 1):
                    for which in (1, 2, 3):
                        mm(p, which, k, k == 0, False)
                mm(p, 1, KC - 1, False, True)
                mm(p, 3, KC - 1, False, True)
                mm(p, 2, KC - 1, False, True)
                evac(p)

    nc.compile()
    return nc


def get_nc():
    if "nc" not in _CACHE:
        _CACHE["nc"] = _build()
    return _CACHE["nc"]


def make_in_maps(input_real, input_imag, weight):
    input_real = np.asarray(input_real)
    input_imag = np.asarray(input_imag)
    weight = np.asarray(weight, dtype=np.float32)
    # pack [S, D] -> [P, KC*D]: row p holds chunks k=0..KC-1 concatenated
    r16 = (
        input_real.astype(np.float16)
        .reshape(B, KC, P, D)
        .transpose(0, 2, 1, 3)
        .reshape(B, P, KC * D)
    )
    i16 = (
        input_imag.astype(np.float16)
        .reshape(B, KC, P, D)
        .transpose(0, 2, 1, 3)
        .reshape(B, P, KC * D)
    )
    # [B, P, KC]: col k = sqrt(w) for chunk k
    s_pack = np.sqrt(weight).astype(np.float32).reshape(B, KC, P).transpose(0, 2, 1)
    return [
        {
            "r_in": np.ascontiguousarray(r16[b]),
            "i_in": np.ascontiguousarray(i16[b]),
            "s_in": np.ascontiguousarray(s_pack[b]),
        }
        for b in range(B)
    ]


def unshard_single(o_np, oin_np, oind_np):
    """fp16 device outputs -> full fp32 [D,D] pair, mirroring the skipped
    strictly-lower blocks (pure transpose copies of device-computed data)."""
    o_np = np.asarray(o_np)
    out_r = o_np[:, 0, :].astype(np.float32)
    out_i = o_np[:, 1, :].astype(np.float32)
    oind = np.asarray(oind_np).astype(np.float32)
    # in-diagonal-block mirrors (out_r symmetric, out_i antisymmetric with
    # the negation already applied on device into oind)
    for b0, a_off, b_off in ((0, 0, 256), (NW, 384, 640)):
        out_r[b0 + P : b0 + NW, b0 : b0 + P] = out_r[b0 : b0 + P, b0 + P : b0 + NW].T
        out_r[b0 + 2 * P : b0 + NW, b0 + P : b0 + 2 * P] = (
            out_r[b0 + P : b0 + 2 * P, b0 + 2 * P : b0 + NW].T
        )
        out_i[b0 + P : b0 + NW, b0 : b0 + P] = oind[:, a_off : a_off + 2 * P].T
        out_i[b0 + 2 * P : b0 + NW, b0 + P : b0 + 2 * P] = (
            oind[:, b_off : b_off + P].T
        )
    # big lower-left 384-block mirrors
    out_r[NW:D, 0:NW] = out_r[0:NW, NW:D].T
    out_i[NW:D, 0:NW] = np.asarray(oin_np).astype(np.float32).T
    return out_r, out_i


def run(input_real, input_imag, weight, **spmd_kwargs):
    nc = get_nc()
    res = bass_utils.run_bass_kernel_spmd(
        nc,
        make_in_maps(input_real, input_imag, weight),
        core_ids=list(range(N_CORES)),
        **spmd_kwargs,
    )
    outs = [
        unshard_single(
            res.results[b]["o_out"], res.results[b]["oin_out"],
            res.results[b]["oind_out"],
        )
        for b in range(B)
    ]
    out_r = np.stack([o[0] for o in outs])
    out_i = np.stack([o[1] for o in outs])
    return (out_r, out_i), res


def kernel(input_real, input_imag, weight):
    (out_r, out_i), _ = run(input_real, input_imag, weight)
    return (out_r, out_i)
